# revision 1
# baseline (speedup 1.0000x reference)
"""Trainium2 Bass kernel for nn_Net_5334349382149 (4-layer GATv2 GNN + config MLP).

Sharding: destination-node partition of edges across 8 cores (2500 nodes/core),
per-layer AllGather of source features, one tiny stats AllReduce per InstanceNorm.

Host-side prep (sanctioned: sharding + parameter constant-folding):
  - edges sorted by dst, partitioned by owner core, padded to uniform
    (dst-tile x TPD) structure; int16 gather indices in dma_gather layout
  - embedding max_norm renorm + feature normalization folded into early_W1/T1
  - GATv2 `att` vector folded into Wl/Wr with a sign-split channel permutation
    so per-edge scores become two fused Relu+accum ops
  - config feature normalization folded into late_W1

Device algorithm per GAT layer (per core):
  xl' = x @ Wl'' + bl''  (att-scaled, channel-major matmuls)
  AllGather(xl') -> xl_full in DRAM;  xr' local in DRAM
  per 128-edge tile: dma_gather rows of xl_full[src], xr'[dst];
    h' = XL + XR (fused sum-accum), e = 0.2*sum(h') + 0.8*(relu_A - relu_B),
    w = exp(e); one-hot(dst)*w matmul-accumulated into PSUM -> segment
    softmax-weighted sums without explicit alpha
  out = (seg/segw)*recip_att + bias; transpose to channel-major; global stats
  AllReduce; x = gelu((out-mu)/sigma)
"""
import os
import sys
import numpy as np

for p in ("/opt/trn_rl_repo", "/opt/pypackages"):
    if p not in sys.path and os.path.isdir(p):
        sys.path.append(p)

import concourse.bass as bass
import concourse.tile as tile
from concourse import bacc, mybir
from concourse.masks import make_identity
from concourse.bass_utils import run_bass_kernel_spmd

F32 = mybir.dt.float32
GDT = mybir.dt.bfloat16
I16 = mybir.dt.int16
I32 = mybir.dt.int32
AF = mybir.ActivationFunctionType
ALU = mybir.AluOpType
AX = mybir.AxisListType

NCORES = 8
HID = 256
EMB = 128
OPS = 120
OPF = 140
CF = 24


class Cfg:
    def __init__(self, NS, NP, TPD, C, CP, nA):
        self.NS = NS            # real nodes per core
        self.NP = NP            # padded nodes per core (mult of 128)
        self.TPD = TPD          # edge tiles per dst tile
        self.C = C              # real configs
        self.CP = CP            # padded configs (mult of 128)
        self.nA = tuple(nA)     # per-layer count of att>=0 channels
        self.NDT = NP // 128    # dst tiles per core
        self.L = self.NDT * TPD * 128   # edge slots per core
        self.N = NS * NCORES    # total real nodes
        self.NPG = NP * NCORES  # padded global rows

    def key(self):
        return (self.NS, self.NP, self.TPD, self.C, self.CP, self.nA)


# ---------------------------------------------------------------------------
# host preprocessing
# ---------------------------------------------------------------------------

def _wrap_idx(idx, pad_to=None):
    """int32 array -> int16 dma_gather layout [128, n/16] (16-wrapped, 8x replicated)."""
    n = len(idx)
    assert n % 16 == 0
    w = idx.astype(np.int16).reshape(n // 16, 16).T          # [16, n/16]
    return np.tile(w, (8, 1))                                # [128, n/16]


def host_prep(d, cfg=None):
    f32 = np.float32
    N_IN = int(np.asarray(d['node_feat']).shape[0])
    E_IN = int(np.asarray(d['edge_index']).shape[1])
    C_IN = int(np.asarray(d['config_feat']).shape[0])

    # ---- parameter folding ----
    tbl = np.asarray(d['embed_table'], f32)
    nrm = np.sqrt((tbl * tbl).sum(-1, keepdims=True))
    tbl = tbl * np.minimum(1.0, 1.0 / (nrm + 1e-7))
    W1 = np.asarray(d['early_W1'], f32)
    T1 = (tbl @ W1[:EMB]).astype(f32)                        # [OPS, 256]
    inv_std = (1.0 / (np.asarray(d['node_feat_std'], f32) + 1e-4)).astype(f32)
    w1b = (W1[EMB:] * inv_std[:, None]).astype(f32)          # [OPF, 256]
    b0 = (-(np.asarray(d['node_feat_mean'], f32) * inv_std) @ W1[EMB:]).astype(f32)

    perms, nAs = [], []
    rho = np.arange(HID)
    wl_l, wr_l, bl_l, br_l, gb_l, ratt_l = [], [], [], [], [], []
    for i in range(4):
        att = np.asarray(d['gat_att'][i], f32)
        pos = np.where(att >= 0)[0]
        neg = np.where(att < 0)[0]
        perm = np.concatenate([pos, neg])
        nAs.append(len(pos))
        assert np.abs(att[perm]).min() > 1e-12
        wl_l.append((np.asarray(d['gat_Wl'][i], f32)[rho][:, perm]
                     * att[perm][None, :]).astype(f32))
        wr_l.append((np.asarray(d['gat_Wr'][i], f32)[rho][:, perm]
                     * att[perm][None, :]).astype(f32))
        bl_l.append(((np.asarray(d['gat_bl'][i], f32) * att)[perm]).astype(f32))
        br_l.append(((np.asarray(d['gat_br'][i], f32) * att)[perm]).astype(f32))
        gb_l.append(np.asarray(d['gat_bias'][i], f32)[perm].astype(f32))
        ratt_l.append((1.0 / att[perm]).astype(f32))
        perms.append(perm)
        rho = perm

    cf_inv = (1.0 / (np.asarray(d['config_feat_std'], f32) + 1e-4)).astype(f32)
    LW1 = np.asarray(d['late_W1'], f32)
    w1c = (LW1[:CF] * cf_inv[:, None]).astype(f32)
    bc0 = (-(np.asarray(d['config_feat_mean'], f32) * cf_inv) @ LW1[:CF]).astype(f32)
    w1p = LW1[CF:][perms[3]].astype(f32)

    # ---- edge sharding ----
    NS = N_IN // NCORES
    NP_ = ((NS + 127) // 128) * 128
    ei = np.asarray(d['edge_index']).astype(np.int64)
    src = np.concatenate([ei[0], np.arange(N_IN)])
    dst = np.concatenate([ei[1], np.arange(N_IN)])
    owner = dst // NS
    loc = dst % NS
    src_pad = (src // NS) * NP_ + (src % NS)
    NDT = NP_ // 128
    per_core_raw = []
    tpd = 1
    for k in range(NCORES):
        m = owner == k
        sk, lk = src_pad[m], loc[m]
        o = np.argsort(lk, kind='stable')
        sk, lk = sk[o], lk[o]
        cnt = np.bincount(lk // 128, minlength=NDT)
        tpd = max(tpd, int(np.ceil(cnt.max() / 128)))
        per_core_raw.append((sk, lk, cnt))
    cfg = Cfg(NS, NP_, tpd, C_IN, ((C_IN + 127) // 128) * 128, nAs)
    L = cfg.L

    per_core = []
    for k in range(NCORES):
        sk, lk, cnt = per_core_raw[k]
        src_g = np.zeros(L, np.int32)
        dst_g = np.zeros(L, np.int32)
        dst_rel = np.full(L, -1000.0, f32)
        off = 0
        for t in range(NDT):
            c = int(cnt[t])
            base = t * cfg.TPD * 128
            src_g[base:base + c] = sk[off:off + c]
            dst_g[base:base + c] = lk[off:off + c]
            dst_rel[base:base + c] = (lk[off:off + c] - t * 128).astype(f32)
            off += c
        per_core.append(dict(src_g=src_g, dst_g=dst_g, dst_rel=dst_rel))

    # ---- per-core input maps ----
    nf = np.asarray(d['node_feat'], f32)
    opc = np.asarray(d['node_opcode']).astype(np.int32)
    cfp = np.zeros((cfg.CP, CF), f32)
    cfp[:C_IN] = np.asarray(d['config_feat'], f32)

    NSV = 18
    sv = np.zeros((HID, NSV), f32)
    sv[:, 0] = b0
    for i in range(4):
        sv[:, 1 + 4 * i] = bl_l[i]
        sv[:, 2 + 4 * i] = br_l[i]
        sv[:, 3 + 4 * i] = gb_l[i]
        sv[:, 4 + 4 * i] = ratt_l[i]
    sv[:, 17] = bc0

    shared = {
        't1': T1,
        'w1ba': w1b[:128], 'w1bb': w1b[128:],
        'w2a': np.asarray(d['early_W2'], f32)[:128],
        'w2b': np.asarray(d['early_W2'], f32)[128:],
        'w1c': w1c,
        'w1pa': w1p[:128], 'w1pb': w1p[128:],
        'w2la': np.asarray(d['late_W2'], f32)[:128],
        'w2lb': np.asarray(d['late_W2'], f32)[128:],
        'predw': np.asarray(d['pred_W'], f32),
        'predb': np.asarray(d['pred_b'], f32).reshape(1, 1),
        'sv_lo': sv[:128].copy(), 'sv_hi': sv[128:].copy(),
        'cf': cfp,
    }
    for i in range(4):
        shared[f'wl{i}a'] = wl_l[i][:128]
        shared[f'wl{i}b'] = wl_l[i][128:]
        shared[f'wr{i}a'] = wr_l[i][:128]
        shared[f'wr{i}b'] = wr_l[i][128:]

    in_maps = []
    for k in range(NCORES):
        e = per_core[k]
        nfk = np.zeros((cfg.NP, OPF), f32)
        nfk[:NS] = nf[k * NS:(k + 1) * NS]
        ok = np.zeros(cfg.NP, np.int32)
        ok[:NS] = opc[k * NS:(k + 1) * NS]
        m = dict(shared)
        m['nf'] = nfk
        m['opidx'] = _wrap_idx(ok)
        m['srcidx'] = _wrap_idx(e['src_g'])
        m['dstidx'] = _wrap_idx(e['dst_g'])
        m['dstrel'] = e['dst_rel'].reshape(cfg.NDT * cfg.TPD, 128).T.copy()
        in_maps.append(m)
    return cfg, in_maps


# ---------------------------------------------------------------------------
# program builder
# ---------------------------------------------------------------------------

def build_program(cfg: Cfg):
    nc = bacc.Bacc("TRN2", target_bir_lowering=False, debug=False,
                   num_devices=NCORES)
    NP_, NS, TPD, NDT, L = cfg.NP, cfg.NS, cfg.TPD, cfg.NDT, cfg.L
    NT = NDT                       # 128-node tiles per core
    REPL = [[list(range(NCORES))][0]]

    def din(name, shape, dt=F32):
        return nc.dram_tensor(name, list(shape), dt, kind="ExternalInput")

    # ---- external inputs ----
    nf_d = din('nf', (NP_, OPF))
    t1_d = din('t1', (OPS, HID))
    opidx_d = din('opidx', (128, NP_ // 16), I16)
    srcidx_d = din('srcidx', (128, L // 16), I16)
    dstidx_d = din('dstidx', (128, L // 16), I16)
    dstrel_d = din('dstrel', (128, NDT * TPD))
    w1ba_d = din('w1ba', (128, HID))
    w1bb_d = din('w1bb', (OPF - 128, HID))
    w2_d = [din('w2a', (128, HID)), din('w2b', (HID - 128, HID))]
    wl_d = [[din(f'wl{i}a', (128, HID)), din(f'wl{i}b', (128, HID))] for i in range(4)]
    wr_d = [[din(f'wr{i}a', (128, HID)), din(f'wr{i}b', (128, HID))] for i in range(4)]
    w1c_d = din('w1c', (CF, HID))
    w1p_d = [din('w1pa', (128, HID)), din('w1pb', (128, HID))]
    w2l_d = [din('w2la', (128, 128)), din('w2lb', (128, 128))]
    predw_d = din('predw', (128, 1))
    predb_d = din('predb', (1, 1))
    sv_d = [din('sv_lo', (128, 18)), din('sv_hi', (128, 18))]
    cf_d = din('cf', (cfg.CP, CF))
    out_d = nc.dram_tensor('out', [1, cfg.CP], F32, kind="ExternalOutput")

    # ---- internal DRAM ----
    xl_own = [nc.dram_tensor(f'xl_own{i}', [NP_, HID], GDT) for i in range(4)]
    xr_own = [nc.dram_tensor(f'xr_own{i}', [NP_, HID], GDT) for i in range(4)]
    xl_full = [nc.dram_tensor(f'xl_full{i}', [cfg.NPG, HID], GDT,
                              addr_space="Shared") for i in range(4)]
    ar_in = [nc.dram_tensor(f'ar_in{i}', [128, 4], F32) for i in range(6)]
    ar_out = [nc.dram_tensor(f'ar_out{i}', [128, 4], F32, addr_space="Shared")
              for i in range(6)]
    pool_in = nc.dram_tensor('pool_in', [128, 4], F32)
    pool_out = nc.dram_tensor('pool_out', [128 * NCORES, 4], F32,
                              addr_space="Shared")

    with tile.TileContext(nc) as tc, __import__('contextlib').ExitStack() as ctx:
        const = ctx.enter_context(tc.tile_pool(name="const", bufs=1))
        big = ctx.enter_context(tc.tile_pool(name="big", bufs=1))
        work = ctx.enter_context(tc.tile_pool(name="work", bufs=3))
        col = ctx.enter_context(tc.tile_pool(name="col", bufs=6))
        psum = ctx.enter_context(tc.tile_pool(name="psum", bufs=2, space="PSUM"))

        # ------ constants ------
        ident = const.tile([128, 128], F32, tag="ident", name="ident")
        make_identity(nc, ident[:])
        iota_i = const.tile([128, 128], I32, tag="iota_i", name="iota_i")
        nc.gpsimd.iota(iota_i[:], pattern=[[1, 128]], base=0, channel_multiplier=0)
        iota_f = const.tile([128, 128], F32, tag="iota_f", name="iota_f")
        nc.vector.tensor_copy(iota_f[:], iota_i[:])
        ones_col = const.tile([128, 1], F32, tag="ones", name="ones")
        nc.gpsimd.memset(ones_col[:], 1.0)
        ones_bf = const.tile([128, 1], GDT, tag="onesbf", name="onesbf")
        nc.gpsimd.memset(ones_bf[:], 1.0)
        iota_bf = const.tile([128, 128], GDT, tag="iota_bf", name="iota_bf")
        nc.vector.tensor_copy(iota_bf[:], iota_i[:])
        zero_col = const.tile([128, 1], F32, tag="zeroc", name="zeroc")
        nc.gpsimd.memset(zero_col[:], 0.0)
        nc.const_aps.aps[(F32, 0.0)] = zero_col[:]
        eps_col = const.tile([128, 1], F32, tag="epsc", name="epsc")
        nc.gpsimd.memset(eps_col[:], 1e-5)

        def load_const(dram, tag):
            t = const.tile(list(dram.shape), dram.dtype, tag=tag)
            nc.sync.dma_start(out=t[:], in_=dram[:])
            return t

        srcidx = load_const(srcidx_d, 'srcidx')
        dstidx = load_const(dstidx_d, 'dstidx')
        opidx = load_const(opidx_d, 'opidx')
        dstrel = load_const(dstrel_d, 'dstrel')
        w1ba = load_const(w1ba_d, 'w1ba')
        w1bb = load_const(w1bb_d, 'w1bb')
        w2 = [load_const(w2_d[j], f'w2{j}') for j in range(2)]
        wl = [[load_const(wl_d[i][j], f'wl{i}{j}') for j in range(2)] for i in range(4)]
        wr = [[load_const(wr_d[i][j], f'wr{i}{j}') for j in range(2)] for i in range(4)]
        w1c = load_const(w1c_d, 'w1c')
        w1p = [load_const(w1p_d[j], f'w1p{j}') for j in range(2)]
        w2l = [load_const(w2l_d[j], f'w2l{j}') for j in range(2)]
        predw = load_const(predw_d, 'predw')
        predb = load_const(predb_d, 'predb')
        sv = [load_const(sv_d[j], f'sv{j}') for j in range(2)]

        # ------ persistent big tiles ------
        raw = [big.tile([128, NP_], F32, tag=f"raw{m}", name=f"raw{m}") for m in range(2)]
        xt = [big.tile([128, NP_], F32, tag=f"x{m}", name=f"x{m}") for m in range(2)]

        # node-free-axis blocks (<=512)
        blocks = [(s, min(s + 512, NP_)) for s in range(0, NP_, 512)]

        def stats_tiles(tagp):
            return [work.tile([128, max(len(blocks), NDT)], F32, tag=f"{tagp}{m}", name=f"{tagp}{m}")
                    for m in range(2)]

        # --- chunked dma_gather (HW limit: <=1024 idxs per instruction) ---
        def gather_rows(out3, in_dram, idx_tile, i0_idx, total, elem):
            """out3: [128, total//128, elem] AP; idx_tile cols start at i0_idx/16."""
            done = 0
            while done < total:
                n = min(1024, total - done)
                nc.gpsimd.dma_gather(
                    out_ap=out3[:, done // 128:(done + n) // 128, :],
                    in_ap=in_dram[:],
                    idxs_ap=idx_tile[:, (i0_idx + done) // 16:(i0_idx + done + n) // 16],
                    num_idxs=n, num_idxs_reg=n, elem_size=elem)
                done += n

        # --- evac with stats accumulation, split at the NS boundary ---
        def evac_block(dst_tile, src_ap, c0, c1, mc, st1, st2, blk_i, scalar1, scalar2):
            """dst_tile[:, c0:c1] = (src op) ...; stats (sum, sumsq) into st1/st2[:, blk_i]
            only over valid columns (< NS)."""
            def one(a, b, accum):
                kw = {}
                if accum:
                    kw['accum_out'] = st1[mc][:, blk_i:blk_i + 1]
                if scalar2 is None:
                    nc.vector.tensor_scalar(dst_tile[:, a:b], src_ap[:, a - c0:b - c0],
                                            scalar1, 0.0, ALU.add, ALU.add, **kw)
                else:
                    nc.vector.tensor_scalar(dst_tile[:, a:b], src_ap[:, a - c0:b - c0],
                                            scalar1, scalar2, ALU.mult, ALU.add, **kw)
                if accum and st2 is not None:
                    sq = work.tile([128, 512], F32, tag="sqscr", name="sqscr")
                    nc.scalar.activation(sq[:, :b - a], dst_tile[:, a:b], AF.Square,
                                         accum_out=st2[mc][:, blk_i:blk_i + 1])
            if c0 >= NS:
                one(c0, c1, False)
            elif c1 <= NS:
                one(c0, c1, True)
            else:
                one(c0, NS, True)
                one(NS, c1, False)

        # --- global stats -> normalize+gelu:  x = gelu((raw - mu) * rstd) ---
        def stats_and_norm(st1, st2, ar_i, ar_o, ntotal, nblk):
            art = work.tile([128, 4], F32, tag="art", name="art")
            for m in range(2):
                nc.vector.tensor_reduce(art[:, 2 * m:2 * m + 1], st1[m][:, :nblk],
                                        AX.X, ALU.add)
                nc.vector.tensor_reduce(art[:, 2 * m + 1:2 * m + 2], st2[m][:, :nblk],
                                        AX.X, ALU.add)
            nc.sync.dma_start(out=ar_i[:], in_=art[:])
            nc.gpsimd.collective_compute(
                "AllReduce", ALU.add, replica_groups=REPL,
                ins=[ar_i[:]], outs=[ar_o[:]])
            arr = work.tile([128, 4], F32, tag="arr", name="arr")
            nc.sync.dma_start(out=arr[:], in_=ar_o[:])
            rs_l, nmr_l = [], []
            for m in range(2):
                mu = col.tile([128, 1], F32, tag="mu", name="mu")
                nc.vector.tensor_scalar(mu[:], arr[:, 2 * m:2 * m + 1],
                                        1.0 / ntotal, None, ALU.mult)
                mu2 = col.tile([128, 1], F32, tag="mu2", name="mu2")
                nc.scalar.activation(mu2[:], mu[:], AF.Square)
                var = col.tile([128, 1], F32, tag="var", name="var")
                nc.vector.scalar_tensor_tensor(var[:], arr[:, 2 * m + 1:2 * m + 2],
                                               1.0 / ntotal, mu2[:],
                                               ALU.mult, ALU.subtract)
                sd = col.tile([128, 1], F32, tag="sd", name="sd")
                nc.scalar.activation(sd[:], var[:], AF.Sqrt, bias=eps_col[:])
                rs = col.tile([128, 1], F32, tag="rs", name="rs")
                nc.vector.reciprocal(rs[:], sd[:])
                nmr = col.tile([128, 1], F32, tag="nmr", name="nmr")
                nc.vector.tensor_scalar(nmr[:], mu[:], rs[:], -1.0, ALU.mult, ALU.mult)
                rs_l.append(rs)
                nmr_l.append(nmr)
            return rs_l, nmr_l

        def norm_gelu(src_tiles, dst_tiles, rs_l, nmr_l):
            for m in range(2):
                nc.scalar.activation(dst_tiles[m][:], src_tiles[m][:], AF.Gelu,
                                     bias=nmr_l[m][:], scale=rs_l[m][:])

        # =================== early stage ===================
        # load node features (node-major) and transpose to channel-major
        early = tc.alloc_tile_pool(name="early", bufs=1)
        nfTa = early.tile([128, NP_], F32, tag="nfTa", name="nfTa")
        nfTb = early.tile([OPF - 128, NP_], F32, tag="nfTb", name="nfTb")
        nf_nm = early.tile([128, NT * OPF], F32, tag="nf_nm", name="nf_nm")
        nc.sync.dma_start(
            out=nf_nm[:].rearrange("p (t c) -> p t c", c=OPF),
            in_=nf_d[:].rearrange("(t p) c -> p t c", p=128))
        emb_nm = early.tile([128, NT * HID], F32, tag="emb_nm", name="emb_nm")
        gather_rows(emb_nm[:].rearrange("p (t c) -> p t c", c=HID),
                    t1_d, opidx, 0, NP_, HID)

        for t in range(NT):
            ps = psum.tile([128, 128], F32, tag="tr", name="tr")
            nc.tensor.matmul(ps[:], lhsT=nf_nm[:, t * OPF:t * OPF + 128],
                             rhs=ident[:], is_transpose=True, start=True, stop=True)
            nc.vector.tensor_copy(nfTa[:, t * 128:(t + 1) * 128], ps[:])
            ps2 = psum.tile([128, 128], F32, tag="tr", name="tr")
            nc.tensor.matmul(ps2[:OPF - 128, :], lhsT=nf_nm[:, t * OPF + 128:(t + 1) * OPF],
                             rhs=ident[:], is_transpose=True, start=True, stop=True)
            nc.vector.tensor_copy(nfTb[:, t * 128:(t + 1) * 128], ps2[:OPF - 128, :])

        # early layer 1: raw = nf @ w1b + T1[op] + b0   (channel-major)
        st1 = stats_tiles("e1s1")
        st2 = stats_tiles("e1s2")
        for mc in range(2):
            for bi, (s, e) in enumerate(blocks):
                w = e - s
                ps = psum.tile([128, 512], F32, tag="mm", name="mm")
                nc.tensor.matmul(ps[:, :w], lhsT=w1ba[:, mc * 128:(mc + 1) * 128],
                                 rhs=nfTa[:, s:e], start=True, stop=False)
                nc.tensor.matmul(ps[:, :w], lhsT=w1bb[:, mc * 128:(mc + 1) * 128],
                                 rhs=nfTb[:, s:e], start=False, stop=False)
                ntile = w // 128
                for tt in range(ntile):
                    gt = (s // 128) + tt
                    nc.tensor.matmul(
                        ps[:, tt * 128:(tt + 1) * 128],
                        lhsT=emb_nm[:, gt * HID + mc * 128: gt * HID + (mc + 1) * 128],
                        rhs=ident[:], is_transpose=True, start=False,
                        stop=(tt == ntile - 1))
                evac_block(raw[mc], ps[:, :w], s, e, mc, st1, st2, bi,
                           sv[mc][:, 0:1], None)
        early.release()
        rs_l, nmr_l = stats_and_norm(st1, st2, ar_in[0], ar_out[0], cfg.N, len(blocks))
        norm_gelu(raw, xt, rs_l, nmr_l)

        # early layer 2: raw = x @ w2
        st1 = stats_tiles("e2s1")
        st2 = stats_tiles("e2s2")
        for mc in range(2):
            for bi, (s, e) in enumerate(blocks):
                w = e - s
                ps = psum.tile([128, 512], F32, tag="mm", name="mm")
                for kc in range(2):
                    nc.tensor.matmul(ps[:, :w], lhsT=w2[kc][:, mc * 128:(mc + 1) * 128],
                                     rhs=xt[kc][:, s:e], start=(kc == 0),
                                     stop=(kc == 1))
                evac_block(raw[mc], ps[:, :w], s, e, mc, st1, st2, bi, 0.0, None)
        rs_l, nmr_l = stats_and_norm(st1, st2, ar_in[1], ar_out[1], cfg.N, len(blocks))
        norm_gelu(raw, xt, rs_l, nmr_l)

        # =================== GAT layers ===================
        gat = tc.alloc_tile_pool(name="gath", bufs=2)
        for li in range(4):
            nA = cfg.nA[li]
            # xl' / xr' (channel-major matmul + bias), transpose to node-major,
            # store to DRAM -- streamed per 512-node block
            for (wmat, dram, bcol) in ((wl[li], xl_own[li], 1 + 4 * li),
                                       (wr[li], xr_own[li], 2 + 4 * li)):
                for (s, e) in blocks:
                    w = e - s
                    blk = []
                    for mc in range(2):
                        ps = psum.tile([128, 512], F32, tag="mm", name="mm")
                        for kc in range(2):
                            nc.tensor.matmul(
                                ps[:, :w], lhsT=wmat[kc][:, mc * 128:(mc + 1) * 128],
                                rhs=xt[kc][:, s:e], start=(kc == 0), stop=(kc == 1))
                        b = work.tile([128, 512], F32, tag=f"xblk{mc}",
                                      name=f"xblk{mc}")
                        nc.vector.tensor_scalar(
                            b[:, :w], ps[:, :w],
                            sv[mc][:, bcol:bcol + 1], None, ALU.add)
                        blk.append(b)
                    for tt in range(w // 128):
                        nm = work.tile([128, HID], GDT, tag="nm", name="nm")
                        for mc in range(2):
                            ps2 = psum.tile([128, 128], F32, tag="tr", name="tr")
                            nc.tensor.matmul(ps2[:],
                                             lhsT=blk[mc][:, tt * 128:(tt + 1) * 128],
                                             rhs=ident[:], is_transpose=True,
                                             start=True, stop=True)
                            nc.vector.tensor_copy(nm[:, mc * 128:(mc + 1) * 128],
                                                  ps2[:])
                        nc.sync.dma_start(
                            out=dram[s + tt * 128:s + (tt + 1) * 128, :], in_=nm[:])
            nc.gpsimd.collective_compute(
                "AllGather", ALU.bypass, replica_groups=REPL,
                ins=[xl_own[li][:]], outs=[xl_full[li][:]])

            # ---- edge processing ----
            st1 = stats_tiles("gs1")
            st2 = stats_tiles("gs2")
            for g in range(NDT):
                xlg = gat.tile([128, TPD * HID], GDT, tag="xlg", name="xlg")
                xrg = gat.tile([128, TPD * HID], GDT, tag="xrg", name="xrg")
                i0 = g * TPD * 128
                gather_rows(xlg[:].rearrange("p (t c) -> p t c", c=HID),
                            xl_full[li], srcidx, i0, TPD * 128, HID)
                gather_rows(xrg[:].rearrange("p (t c) -> p t c", c=HID),
                            xr_own[li], dstidx, i0, TPD * 128, HID)
                ps256 = psum.tile([128, HID], F32, tag="e256", name="e256")
                ps1 = psum.tile([128, 1], F32, tag="e1", name="e1")
                racols = col.tile([128, TPD], F32, tag="racols", name="racols")
                rbcols = col.tile([128, TPD], F32, tag="rbcols", name="rbcols")
                hscols = col.tile([128, TPD], F32, tag="hscols", name="hscols")
                for j in range(TPD):
                    XL = xlg[:, j * HID:(j + 1) * HID]
                    XR = xrg[:, j * HID:(j + 1) * HID]
                    ht = work.tile([128, HID], GDT, tag="ht", name="ht")
                    nc.vector.scalar_tensor_tensor(ht[:], XL, 1.0, XR,
                                                   ALU.mult, ALU.add,
                                                   accum_out=hscols[:, j:j + 1])
                    lr = work.tile([128, HID], GDT, tag="lr", name="lr")
                    nc.scalar.activation(lr[:, :nA], ht[:, :nA], AF.Relu,
                                         accum_out=racols[:, j:j + 1])
                    nc.scalar.activation(lr[:, nA:], ht[:, nA:], AF.Relu,
                                         scale=-1.0, accum_out=rbcols[:, j:j + 1])
                # e = 0.8*((rA - rB) + 0.25*hsum); w = exp(e)
                tcols = col.tile([128, TPD], F32, tag="tcols", name="tcols")
                nc.vector.tensor_tensor(tcols[:], racols[:], rbcols[:], ALU.subtract)
                ecols = col.tile([128, TPD], F32, tag="ecols", name="ecols")
                nc.vector.scalar_tensor_tensor(ecols[:], hscols[:], 0.25, tcols[:],
                                               ALU.mult, ALU.add)
                wcols = col.tile([128, TPD], F32, tag="wcols", name="wcols")
                nc.scalar.activation(wcols[:], ecols[:], AF.Exp, scale=0.8)
                for j in range(TPD):
                    XL = xlg[:, j * HID:(j + 1) * HID]
                    oh = work.tile([128, 128], GDT, tag="oh", name="oh")
                    et = g * TPD + j
                    nc.vector.tensor_scalar(oh[:], iota_bf[:],
                                            dstrel[:, et:et + 1], wcols[:, j:j + 1],
                                            ALU.is_equal, ALU.mult)
                    nc.tensor.matmul(ps256[:], lhsT=oh[:], rhs=XL,
                                     start=(j == 0), stop=(j == TPD - 1))
                    nc.tensor.matmul(ps1[:], lhsT=oh[:], rhs=ones_bf[:],
                                     start=(j == 0), stop=(j == TPD - 1))
                # ---- per dst-tile evacuation ----
                dcol = col.tile([128, 1], F32, tag="dcol", name="dcol")
                nc.vector.tensor_scalar(dcol[:], ps1[:], 1e-16, None, ALU.add)
                rcol = col.tile([128, 1], F32, tag="rcol", name="rcol")
                nc.vector.reciprocal(rcol[:], dcol[:])
                msg = work.tile([128, HID], F32, tag="msg", name="msg")
                nc.vector.tensor_scalar(msg[:], ps256[:], rcol[:], None, ALU.mult)
                for mc in range(2):
                    ps = psum.tile([128, 128], F32, tag="tr", name="tr")
                    nc.tensor.matmul(ps[:], lhsT=msg[:, mc * 128:(mc + 1) * 128],
                                     rhs=ident[:], is_transpose=True,
                                     start=True, stop=True)
                    evac_block(raw[mc], ps[:], g * 128, (g + 1) * 128, mc,
                               st1, st2, g, sv[mc][:, 4 + 4 * li:5 + 4 * li],
                               sv[mc][:, 3 + 4 * li:4 + 4 * li])
            rs_l, nmr_l = stats_and_norm(st1, st2, ar_in[2 + li], ar_out[2 + li],
                                         cfg.N, NDT)
            norm_gelu(raw, xt, rs_l, nmr_l)

        gat.release()
        # =================== pooling ===================
        pt = work.tile([128, 4], F32, tag="pt", name="pt")
        for m in range(2):
            nc.vector.tensor_reduce(pt[:, m:m + 1], xt[m][:, :NS], AX.X, ALU.add)
            nc.vector.tensor_reduce(pt[:, 2 + m:3 + m], xt[m][:, :NS], AX.X, ALU.max)
        nc.sync.dma_start(out=pool_in[:], in_=pt[:])
        nc.gpsimd.collective_compute(
            "AllGather", ALU.bypass, replica_groups=REPL,
            ins=[pool_in[:]], outs=[pool_out[:]])
        pg = work.tile([128, NCORES * 4], F32, tag="pg", name="pg")
        nc.sync.dma_start(out=pg[:].rearrange("p (k v) -> p k v", v=4),
                          in_=pool_out[:].rearrange("(k p) v -> p k v", p=128))
        pg3 = pg[:].rearrange("p (k v) -> p k v", v=4)
        pool_c = []
        for m in range(2):
            s_ = col.tile([128, 1], F32, tag="psum_c", name="psum_c")
            nc.vector.tensor_reduce(s_[:], pg3[:, :, m], AX.X, ALU.add)
            mx = col.tile([128, 1], F32, tag="pmax_c", name="pmax_c")
            nc.vector.tensor_reduce(mx[:], pg3[:, :, 2 + m], AX.X, ALU.max)
            pc = col.tile([128, 1], F32, tag="pool_c", name="pool_c")
            nc.vector.scalar_tensor_tensor(pc[:], s_[:], 1.0 / cfg.N, mx[:],
                                           ALU.mult, ALU.add)
            pool_c.append(pc)

        # =================== late MLP (replicated) ===================
        cblocks = [(s, min(s + 512, cfg.CP)) for s in range(0, cfg.CP, 512)]
        NCT = cfg.CP // 128
        cf_nm = work.tile([128, NCT * CF], F32, tag="cf_nm", name="cf_nm")
        nc.sync.dma_start(out=cf_nm[:].rearrange("p (t c) -> p t c", c=CF),
                          in_=cf_d[:].rearrange("(t p) c -> p t c", p=128))
        late = tc.alloc_tile_pool(name="late", bufs=1)
        cfT = late.tile([CF, cfg.CP], F32, tag="cfT", name="cfT")
        for t in range(NCT):
            ps = psum.tile([128, 128], F32, tag="tr", name="tr")
            nc.tensor.matmul(ps[:CF, :], lhsT=cf_nm[:, t * CF:(t + 1) * CF],
                             rhs=ident[:], is_transpose=True, start=True, stop=True)
            nc.vector.tensor_copy(cfT[:, t * 128:(t + 1) * 128], ps[:CF, :])

        # vec1 = w1p^T @ pool  (+ bc0)
        vcol = []
        for mc in range(2):
            ps = psum.tile([128, 1], F32, tag="e1", name="e1")
            for kc in range(2):
                nc.tensor.matmul(ps[:], lhsT=w1p[kc][:, mc * 128:(mc + 1) * 128],
                                 rhs=pool_c[kc][:], start=(kc == 0), stop=(kc == 1))
            v = col.tile([128, 1], F32, tag="vcol", name="vcol")
            nc.vector.scalar_tensor_tensor(v[:], sv[mc][:, 17:18], 1.0, ps[:],
                                           ALU.mult, ALU.add)
            vcol.append(v)

        h1 = [late.tile([128, cfg.CP], F32, tag=f"h1_{m}", name=f"h1_{m}") for m in range(2)]
        h2 = [late.tile([128, cfg.CP], F32, tag=f"h2_{m}", name=f"h2_{m}") for m in range(2)]

        def cfg_stats_norm(src_tiles, dst_tiles, st1, st2, ar_i, ar_o, nblk, two_chunks):
            rs_l, nmr_l = [], []
            for m in range(2 if two_chunks else 1):
                s1 = col.tile([128, 1], F32, tag="cs1", name="cs1")
                s2c = col.tile([128, 1], F32, tag="cs2", name="cs2")
                nc.vector.tensor_reduce(s1[:], st1[m][:, :nblk], AX.X, ALU.add)
                nc.vector.tensor_reduce(s2c[:], st2[m][:, :nblk], AX.X, ALU.add)
                mu = col.tile([128, 1], F32, tag="mu", name="mu")
                nc.vector.tensor_scalar(mu[:], s1[:], 1.0 / cfg.C, None, ALU.mult)
                mu2 = col.tile([128, 1], F32, tag="mu2", name="mu2")
                nc.scalar.activation(mu2[:], mu[:], AF.Square)
                var = col.tile([128, 1], F32, tag="var", name="var")
                nc.vector.scalar_tensor_tensor(var[:], s2c[:], 1.0 / cfg.C, mu2[:],
                                               ALU.mult, ALU.subtract)
                sd = col.tile([128, 1], F32, tag="sd", name="sd")
                nc.scalar.activation(sd[:], var[:], AF.Sqrt, bias=eps_col[:])
                rs = col.tile([128, 1], F32, tag="rs", name="rs")
                nc.vector.reciprocal(rs[:], sd[:])
                nmr = col.tile([128, 1], F32, tag="nmr", name="nmr")
                nc.vector.tensor_scalar(nmr[:], mu[:], rs[:], -1.0, ALU.mult, ALU.mult)
                rs_l.append(rs)
                nmr_l.append(nmr)
            for m in range(2 if two_chunks else 1):
                nc.scalar.activation(dst_tiles[m][:], src_tiles[m][:], AF.Gelu,
                                     bias=nmr_l[m][:], scale=rs_l[m][:])

        # h1 = gelu(cfgnorm(cf @ w1c + vec1))
        st1 = stats_tiles("l1s1")
        st2 = stats_tiles("l1s2")
        for mc in range(2):
            for bi, (s, e) in enumerate(cblocks):
                w = e - s
                ps = psum.tile([128, 512], F32, tag="mm", name="mm")
                nc.tensor.matmul(ps[:, :w], lhsT=w1c[:, mc * 128:(mc + 1) * 128],
                                 rhs=cfT[:, s:e], start=True, stop=True)

                def cone(a, b, accum):
                    kw = {'accum_out': st1[mc][:, bi:bi + 1]} if accum else {}
                    nc.vector.tensor_scalar(h1[mc][:, a:b], ps[:, a - s:b - s],
                                            vcol[mc][:], 0.0, ALU.add, ALU.add, **kw)
                    if accum:
                        sq = work.tile([128, 512], F32, tag="sqscr", name="sqscr")
                        nc.scalar.activation(sq[:, :b - a], h1[mc][:, a:b], AF.Square,
                                             accum_out=st2[mc][:, bi:bi + 1])
                if s >= cfg.C:
                    cone(s, e, False)
                elif e <= cfg.C:
                    cone(s, e, True)
                else:
                    cone(s, cfg.C, True)
                    cone(cfg.C, e, False)
        cfg_stats_norm(h1, h1, st1, st2, None, None, len(cblocks), True)

        # h2 = gelu(cfgnorm(h1 @ w2l))   (HID//2 = 128 out channels -> mc=0 only)
        st1 = stats_tiles("l2s1")
        st2 = stats_tiles("l2s2")
        for bi, (s, e) in enumerate(cblocks):
            w = e - s
            ps = psum.tile([128, 512], F32, tag="mm", name="mm")
            for kc in range(2):
                nc.tensor.matmul(ps[:, :w], lhsT=w2l[kc][:], rhs=h1[kc][:, s:e],
                                 start=(kc == 0), stop=(kc == 1))

            def done(a, b, accum):
                kw = {'accum_out': st1[0][:, bi:bi + 1]} if accum else {}
                nc.vector.tensor_scalar(h2[0][:, a:b], ps[:, a - s:b - s],
                                        0.0, 0.0, ALU.add, ALU.add, **kw)
                if accum:
                    sq = work.tile([128, 512], F32, tag="sqscr", name="sqscr")
                    nc.scalar.activation(sq[:, :b - a], h2[0][:, a:b], AF.Square,
                                         accum_out=st2[0][:, bi:bi + 1])
            if s >= cfg.C:
                done(s, e, False)
            elif e <= cfg.C:
                done(s, e, True)
            else:
                done(s, cfg.C, True)
                done(cfg.C, e, False)
        cfg_stats_norm(h2, h2, st1, st2, None, None, len(cblocks), False)

        # pred: out = h2^T @ predw + predb
        outsb = work.tile([1, cfg.CP], F32, tag="outsb", name="outsb")
        for (s, e) in cblocks:
            w = e - s
            ps = psum.tile([1, 512], F32, tag="mm", name="predps")
            nc.tensor.matmul(ps[:, :w], lhsT=predw[:], rhs=h2[0][:, s:e],
                             start=True, stop=True)
            nc.vector.tensor_scalar(outsb[:, s:e], ps[:, :w], predb[:],
                                    None, ALU.add)
        nc.sync.dma_start(out=out_d[:], in_=outsb[:])
        late.release()

    nc.compile()
    return nc


# ---------------------------------------------------------------------------
# entry point
# ---------------------------------------------------------------------------

_prog_cache = {}


def kernel(**inputs) -> np.ndarray:
    cfg, in_maps = host_prep(inputs)
    key = cfg.key()
    if key not in _prog_cache:
        _prog_cache[key] = build_program(cfg)
    nc = _prog_cache[key]
    res = run_bass_kernel_spmd(nc, in_maps, list(range(NCORES)))
    out = np.asarray(res.results[0]['out']).reshape(-1)[:cfg.C]
    return out.astype(np.float32)



# revision 8
# speedup vs baseline: 4171.8094x; 4171.8094x over previous
"""Trainium2 Bass kernel for nn_Net_5334349382149.

Mathematical reduction: the graph/pool branch cancels exactly.

The network ends with
    x_late = concat([cf_norm, broadcast(pool)], axis=1)      # [C, 280]
    h = gelu(_mynorm(x_late @ late_W1, axis=0))              # norm over C
    h = gelu(_mynorm(h @ late_W2, axis=0))
    out = h @ pred_W + pred_b
where `pool` (the GNN read-out) is one vector broadcast to every config row.
`_mynorm(z, 0)` subtracts the per-channel mean over the config axis and the
variance is shift-invariant, so ANY row-constant contribution to z — the
entire pool @ late_W1[24:] term, and likewise the -config_mean term of
cf_norm — cancels identically, for every input. The graded output therefore
equals the config-only MLP below (verified: rel err ~3e-6 vs the full
reference in fp32; ~3e-4 on device with fp32r matmuls).

Device kernel per core (all 8 cores replicate; no collectives):
    h1T = late_W1[:24]'^T @ cfT           (channel-major, fp32r matmuls)
    h1  = gelu((h1T - mu)/sigma)          per-channel stats over 1000 configs
    h2T = late_W2^T @ h1
    h2  = gelu((h2T - mu)/sigma)
    out = pred_W^T @ h2 + pred_b

Perf notes:
  - inputs arrive in two packed DMA blobs (one per engine queue)
  - mean/var via DVE bn_stats/bn_aggr over valid columns only
  - 1/sqrt(var+eps) via Quake bit-trick + 2 Newton steps, all DVE int/f32
    ALU ops (verified 5e-6 rel err on hw) — the Act engine runs ONLY Gelu,
    so exactly one act-table load, hoisted under the input DMA by a dummy
    gelu at t0
"""
import os
import sys
import numpy as np

for p in ("/opt/trn_rl_repo", "/opt/pypackages"):
    if p not in sys.path and os.path.isdir(p):
        sys.path.append(p)

import concourse.bass as bass
import concourse.tile as tile
from concourse import bacc, mybir
from concourse.bass_utils import run_bass_kernel_spmd

F32 = mybir.dt.float32
F32R = mybir.dt.float32r
I32 = mybir.dt.int32
AF = mybir.ActivationFunctionType
ALU = mybir.AluOpType
AX = mybir.AxisListType

NCORES = 8
CF = 24
HID = 256
QK = 0x5f3759df


def host_prep(d):
    f32 = np.float32
    cf = np.asarray(d['config_feat'], f32)
    C = cf.shape[0]
    CP = ((C + 127) // 128) * 128
    inv_std = (1.0 / (np.asarray(d['config_feat_std'], f32) + 1e-4)).astype(f32)
    w1c = (np.asarray(d['late_W1'], f32)[:CF] * inv_std[:, None]).astype(f32)
    w2 = np.asarray(d['late_W2'], f32)

    cfw = np.zeros((CF, CP + HID), f32)      # [24, CP] cfT | [24, 256] w1c
    cfw[:, :C] = cf.T
    cfw[:, CP:] = w1c
    w2p = np.zeros((128, 258), f32)          # w2a | w2b | predw | predb
    w2p[:, :128] = w2[:128]
    w2p[:, 128:256] = w2[128:]
    w2p[:, 256:257] = np.asarray(d['pred_W'], f32)
    w2p[0, 257] = np.asarray(d['pred_b'], f32).reshape(())
    m = {'cfw': cfw, 'w2p': w2p}
    return C, CP, [m] * NCORES


def build_program(C, CP):
    nc = bacc.Bacc("TRN2", target_bir_lowering=False, debug=False,
                   num_devices=NCORES)
    blocks = [(s, min(s + 512, CP)) for s in range(0, CP, 512)]
    NB = len(blocks)

    cfw_d = nc.dram_tensor('cfw', [CF, CP + HID], F32R, kind="ExternalInput")
    w2p_d = nc.dram_tensor('w2p', [128, 258], F32R, kind="ExternalInput")
    out_d = nc.dram_tensor('out', [1, CP], F32, kind="ExternalOutput")

    with tile.TileContext(nc) as tc, __import__('contextlib').ExitStack() as ctx:
        const = ctx.enter_context(tc.tile_pool(name="const", bufs=1))
        work = ctx.enter_context(tc.tile_pool(name="work", bufs=1))
        col = ctx.enter_context(tc.tile_pool(name="col", bufs=1))
        psum = ctx.enter_context(tc.tile_pool(name="psum", bufs=1, space="PSUM"))

        cfw = const.tile([CF, CP + HID], F32R, tag="cfw", name="cfw")
        nc.sync.dma_start(out=cfw[:], in_=cfw_d[:])
        w2p = const.tile([128, 258], F32R, tag="w2p", name="w2p")
        nc.gpsimd.dma_start(out=w2p[:], in_=w2p_d[:])
        cfT = cfw[:, :CP]
        w1c = cfw[:, CP:]
        w2 = [w2p[:, :128], w2p[:, 128:256]]
        predw = w2p[:, 256:257]
        predb = w2p[0:1, 257:258].bitcast(F32)

        # dummy gelu at t0: pins the (only) act-table load under the DMA
        eps_col = const.tile([128, 1], F32, tag="epsc", name="epsc")
        nc.gpsimd.memset(eps_col[:], 0.0)
        warm = col.tile([128, 1], F32, tag="warm", name="warm")
        nc.scalar.activation(warm[:], eps_col[:], AF.Gelu)

        h1 = [work.tile([128, CP], F32R, tag=f"h1_{m}", name=f"h1_{m}")
              for m in range(2)]
        h2 = work.tile([128, CP], F32R, tag="h2", name="h2")
        outsb = work.tile([1, CP], F32, tag="outsb", name="outsb")

        def rsqrt_cols(var_ap, nm, width):
            """rs = (var + 1e-5) ** -0.5 entirely on DVE (Quake + 2 Newton)."""
            ve = col.tile([128, width], F32, tag=f"ve_{nm}", name=f"ve_{nm}")
            nc.vector.tensor_scalar(ve[:], var_ap, 1e-5, None, ALU.add)
            yi = col.tile([128, width], I32, tag=f"yi_{nm}", name=f"yi_{nm}")
            nc.vector.tensor_scalar(yi[:], ve[:].bitcast(I32), 1, None,
                                    ALU.arith_shift_right)
            nc.vector.tensor_scalar(yi[:], yi[:], QK, -1, ALU.subtract, ALU.mult)
            y = col.tile([128, width], F32, tag=f"y_{nm}", name=f"y_{nm}")
            nc.vector.tensor_copy(y[:], yi[:].bitcast(F32))
            a = col.tile([128, width], F32, tag=f"qa_{nm}", name=f"qa_{nm}")
            for _ in range(2):
                nc.vector.tensor_tensor(a[:], y[:], y[:], ALU.mult)
                nc.vector.tensor_tensor(a[:], a[:], ve[:], ALU.mult)
                nc.vector.tensor_scalar(a[:], a[:], -0.5, 1.5, ALU.mult, ALU.add)
                nc.vector.tensor_tensor(y[:], y[:], a[:], ALU.mult)
            return y

        def scale_cols(mv_list, nm):
            """[(mean,var) col pairs] -> (rs, nmr) [128, len] tiles."""
            width = len(mv_list)
            mean = col.tile([128, width], F32, tag=f"mean_{nm}", name=f"mean_{nm}")
            var = col.tile([128, width], F32, tag=f"var_{nm}", name=f"var_{nm}")
            for i, mv in enumerate(mv_list):
                nc.vector.tensor_copy(mean[:, i:i + 1], mv[:, 0:1])
                nc.vector.tensor_copy(var[:, i:i + 1], mv[:, 1:2])
            rs = rsqrt_cols(var[:], nm, width)
            nmr = col.tile([128, width], F32, tag=f"nmr_{nm}", name=f"nmr_{nm}")
            nc.vector.scalar_tensor_tensor(nmr[:], mean[:], -1.0, rs[:],
                                           ALU.mult, ALU.mult)
            return rs, nmr

        # ---- layer 1: h1raw = w1c^T @ cfT ----
        ps1t = {}
        bn1 = [work.tile([128, 6 * NB], F32, tag=f"bn1_{m}", name=f"bn1_{m}")
               for m in range(2)]
        for bi, (s, e) in enumerate(blocks):
            for mc in range(2):
                w = e - s
                wv = min(e, C) - s               # valid (non-pad) columns
                ps = psum.tile([128, 512], F32, tag=f"mm{mc}{bi}", name=f"mm{mc}{bi}")
                nc.tensor.matmul(ps[:, :w],
                                 lhsT=w1c[:, mc * 128:(mc + 1) * 128],
                                 rhs=cfT[:, s:e],
                                 start=True, stop=True)
                ps1t[(mc, bi)] = ps
                nc.vector.bn_stats(bn1[mc][:, 6 * bi:6 * bi + 6], ps[:, :wv])
        mv1 = []
        for mc in range(2):
            mv = col.tile([128, 2], F32, tag=f"mv1_{mc}", name=f"mv1_{mc}")
            nc.vector.bn_aggr(mv[:], bn1[mc][:])
            mv1.append(mv)
        rs1, nmr1 = scale_cols(mv1, "h1")
        for bi, (s, e) in enumerate(blocks):
            for mc in range(2):
                nc.scalar.activation(h1[mc][:, s:e], ps1t[(mc, bi)][:, :e - s],
                                     AF.Gelu, bias=nmr1[:, mc:mc + 1],
                                     scale=rs1[:, mc:mc + 1])

        # ---- layer 2: h2raw = w2^T @ h1 ----
        # pad cols of h1 hold gelu(bias) garbage, but bn_stats reads only
        # valid columns and the host discards pad outputs
        ps2t = {}
        bn2 = work.tile([128, 6 * NB], F32, tag="bn2", name="bn2")
        for bi, (s, e) in enumerate(blocks):
            w = e - s
            wv = min(e, C) - s
            ps = psum.tile([128, 512], F32, tag=f"nn{bi}", name=f"nn{bi}")
            for kc in range(2):
                nc.tensor.matmul(ps[:, :w], lhsT=w2[kc], rhs=h1[kc][:, s:e],
                                 start=(kc == 0), stop=(kc == 1))
            ps2t[bi] = ps
            nc.vector.bn_stats(bn2[:, 6 * bi:6 * bi + 6], ps[:, :wv])
        mv2 = col.tile([128, 2], F32, tag="mv2", name="mv2")
        nc.vector.bn_aggr(mv2[:], bn2[:])
        rs2, nmr2 = scale_cols([mv2], "h2")
        for bi, (s, e) in enumerate(blocks):
            nc.scalar.activation(h2[:, s:e], ps2t[bi][:, :e - s],
                                 AF.Gelu, bias=nmr2[:], scale=rs2[:])

        # ---- prediction head ----
        for bi, (s, e) in enumerate(blocks):
            w = e - s
            ps = psum.tile([1, 512], F32, tag=f"pp{bi}", name=f"pp{bi}")
            nc.tensor.matmul(ps[:, :w], lhsT=predw, rhs=h2[:, s:e],
                             start=True, stop=True)
            if bi == 0:   # split the bias adds across two idle engines
                nc.vector.tensor_scalar(outsb[:, s:e], ps[:, :w], predb,
                                        None, ALU.add)
            else:
                nc.scalar.activation(outsb[:, s:e], ps[:, :w], AF.Identity,
                                     bias=predb)
        nc.sync.dma_start(out=out_d[:], in_=outsb[:])

    nc.compile()
    return nc


_prog_cache = {}


def kernel(**inputs) -> np.ndarray:
    C, CP, in_maps = host_prep(inputs)
    key = (C, CP)
    if key not in _prog_cache:
        _prog_cache[key] = build_program(C, CP)
    nc = _prog_cache[key]
    res = run_bass_kernel_spmd(nc, in_maps, list(range(NCORES)))
    out = np.asarray(res.results[0]['out']).reshape(-1)[:C]
    return out.astype(np.float32)
